# revision 1
# baseline (speedup 1.0000x reference)
"""DGQP (distribution-guided quality predictor) Trainium2 Bass kernel.

Full-input contract: kernel(**inputs) takes the unsharded inputs from
setup_inputs() and returns the full (32, 1, 160, 160) float32 output.
Internally: pure data parallel over 8 NeuronCores (4 images each).

Per-image pipeline on one core (H*W = 25600 = 128 q-blocks x 200 f):
  DMA     x[n] (32, 25600) -> SBUF X [128 part=q, free = j*800 + g*200 + f]
  ACT     E = exp(X)  (bf16)  -- softmax without max-subtract (|x| ~ N(0,1))
  GPSIMD  Dn[g,f] = sum_j E   (window-8 reduce, denominators)
  ACT     R = exp(-ln(Dn))    (reciprocal via ln/exp)
  DVE     top-4-of-8 sorted per (g, q, f) via an 18-instruction
          compare-exchange network over the 8 bin slices (bf16, 2x mode)
  DVE     ST = top4 * R  (scaled stats, ch = 4g+jj at stride 200)
  PE      stat transposes [128,16]->[16,128] (is_transpose, 4-way col tiling)
  ACT     PSUM->SBUF copy of transposed stats (bf16)
  PE      mm1: y1 = Wfold.T @ stat  (tile_position 8-way 32x32 tiling;
          mean-channel folded into Wfold; zero rows kill junk partitions)
  ACT/DVE relu(y1 + b1) PSUM->SBUF bf16 (split across both engines)
  PE      mm2: y2 = y1relu.T @ w2cols  (K=128 stationary data, FWL; output
          lands pixel-partitioned [128 q, 2 f-columns])
  ACT     sigmoid(y2 + b2) -> SBUF [128 q, 200 f]
  DMA     -> y[n] (25600,)
"""

import numpy as np
import ml_dtypes
from contextlib import ExitStack

import concourse.bass as bass
import concourse.mybir as mybir
from concourse.tile import TileContext
from concourse import bass_utils

F32 = mybir.dt.float32
BF16 = mybir.dt.bfloat16
AX = mybir.AxisListType
ALU = mybir.AluOpType
AF = mybir.ActivationFunctionType

N_CORES = 8
N_PER = 4          # images per core
C_IN = 32          # channels = 4 groups x 8 bins
HW = 25600         # 160*160
Q = 128            # partition blocks
F = 200            # pixels per partition per image
GF = 800           # 4 groups * 200


def _sl(t, k, w=GF):
    """Flat slice k of width w in the free dim."""
    return t[:, k * w:(k + 1) * w]


def _slots(t, off, a, s, gf=GF):
    """View t[:, off:off+a*s*gf] as (a, s, gf): slot picks with stride a*s."""
    return t[:, off:off + a * s * gf].rearrange(
        "q (a s gf) -> q a s gf", a=a, s=s, gf=gf)


def build_bass(n_img=N_PER, legalize=True):
    nc = bass.Bass("TRN2", target_bir_lowering=False, debug=False)

    x_d = nc.dram_tensor("x", (n_img, C_IN, HW), F32, kind="ExternalInput").ap()
    w1s_d = nc.dram_tensor("w1s", (128, 64), BF16, kind="ExternalInput").ap()
    w2c_d = nc.dram_tensor("w2c", (128, 2), BF16, kind="ExternalInput").ap()
    b1s_d = nc.dram_tensor("b1s", (128, 1), F32, kind="ExternalInput").ap()
    b2s_d = nc.dram_tensor("b2s", (128, 1), F32, kind="ExternalInput").ap()
    idt_d = nc.dram_tensor("idt", (128, 128), BF16, kind="ExternalInput").ap()
    y_d = nc.dram_tensor("y", (n_img, HW), F32, kind="ExternalOutput").ap()

    with TileContext(nc) as tc, ExitStack() as ctx:
        cpool = ctx.enter_context(tc.tile_pool(name="const", bufs=1))
        xpool = ctx.enter_context(tc.tile_pool(name="xin", bufs=2))
        epool = ctx.enter_context(tc.tile_pool(name="exp", bufs=2))
        spool = ctx.enter_context(tc.tile_pool(name="sort", bufs=1))
        dpool = ctx.enter_context(tc.tile_pool(name="den", bufs=2))
        mpool = ctx.enter_context(tc.tile_pool(name="mb", bufs=3))
        zpool = ctx.enter_context(tc.tile_pool(name="z", bufs=3))
        opool = ctx.enter_context(tc.tile_pool(name="out", bufs=2))
        pt_ps = ctx.enter_context(tc.tile_pool(name="pt", bufs=2, space="PSUM"))
        yp_ps = ctx.enter_context(tc.tile_pool(name="yp", bufs=4, space="PSUM"))
        y2_ps = ctx.enter_context(tc.tile_pool(name="y2", bufs=2, space="PSUM"))

        w1s = cpool.tile([128, 64], BF16, tag="w1s")
        w2c = cpool.tile([128, 2], BF16, tag="w2c")
        b1s = cpool.tile([128, 1], F32, tag="b1s")
        b2s = cpool.tile([128, 1], F32, tag="b2s")
        idt = cpool.tile([128, 128], BF16, tag="idt")
        nc.sync.dma_start(out=w1s[:, :], in_=w1s_d)
        nc.sync.dma_start(out=w2c[:, :], in_=w2c_d)
        nc.sync.dma_start(out=b1s[:, :], in_=b1s_d)
        nc.sync.dma_start(out=b2s[:, :], in_=b2s_d)
        nc.sync.dma_start(out=idt[:, :], in_=idt_d)

        for n in range(n_img):
            # X/E free layout is c-major: c*200 + f with c = 8g + j,
            # so the input is a single 3-dim DMA (contiguous 800B runs).
            X = xpool.tile([128, 6400], F32, tag="x")
            nc.sync.dma_start(
                out=X[:, :].rearrange("q (c f) -> q c f", c=32, f=F),
                in_=x_d[n].rearrange("c (q f) -> q c f", q=Q, f=F))

            # exp of all logits (order-preserving for the sort; feeds denom)
            E = epool.tile([128, 6400], BF16, tag="e")
            nc.scalar.activation(out=E[:, :], in_=X[:, :], func=AF.Exp)

            # denominators: sum over the 8 bins via a GpSimd add-tree
            # (m_p + n_p = e_2p + e_2p+1, so reuse the step-1 sort output S)
            Dn = dpool.tile([128, GF], F32, tag="dn")
            LD = dpool.tile([128, GF], F32, tag="ld")
            R = dpool.tile([128, GF], BF16, tag="r")

            # ---- top-4-of-8 sorting network (bf16 slices of 800) ----
            S = spool.tile([128, 6400], BF16, tag="s")
            T = spool.tile([128, 6400], BF16, tag="t")
            Cc = spool.tile([128, 3200], BF16, tag="c")
            # ST holds the scaled stats in ch 0..15 (stride 200); its upper
            # half doubles as the step-4 scratch (Dd) so the 32-wide
            # transposes read fully-initialized junk in ch 16..31.
            ST = spool.tile([128, 6400], BF16, tag="st")
            Dd = ST[:, 3200:6400]

            # E slice for bin j of group g sits at (g*8 + j)*200; scratch
            # tiles S/T/C/D/ST keep slot-major layout (slot*800, g*200+f
            # inside), so only the E-side views are (p, two, g, f) shaped.
            Ev = E[:, :].rearrange("q (g p two f) -> q p two g f",
                                   g=4, p=4, two=2, f=F)
            S04 = S[:, 0:3200].rearrange("q (a g f) -> q a g f",
                                         a=4, g=4, f=F)
            S48 = S[:, 3200:6400].rearrange("q (a g f) -> q a g f",
                                            a=4, g=4, f=F)
            # step1: sorted pairs  m_p = max, n_p = min
            nc.vector.tensor_tensor(out=S04, in0=Ev[:, :, 0],
                                    in1=Ev[:, :, 1], op=ALU.max)
            nc.vector.tensor_tensor(out=S48, in0=Ev[:, :, 0],
                                    in1=Ev[:, :, 1], op=ALU.min)
            # denominator add-tree on GpSimd (reads S, independent of DVE)
            S04f = S[:, 0:3200].rearrange("q (a gf) -> q a gf", a=4, gf=GF)
            S48f = S[:, 3200:6400].rearrange("q (a gf) -> q a gf",
                                             a=4, gf=GF)
            P2 = dpool.tile([128, 3200], F32, tag="p2")
            nc.gpsimd.tensor_tensor(
                out=P2[:, :].rearrange("q (a gf) -> q a gf", a=4, gf=GF),
                in0=S04f, in1=S48f, op=ALU.add)
            nc.gpsimd.tensor_tensor(
                out=P2[:, 0:1600].rearrange("q (a gf) -> q a gf", a=2, gf=GF),
                in0=P2[:, 0:1600].rearrange("q (a gf) -> q a gf", a=2, gf=GF),
                in1=P2[:, 1600:3200].rearrange("q (a gf) -> q a gf",
                                               a=2, gf=GF),
                op=ALU.add)
            nc.gpsimd.tensor_tensor(out=Dn[:, :], in0=P2[:, 0:GF],
                                    in1=P2[:, GF:1600], op=ALU.add)
            nc.scalar.activation(out=LD[:, :], in_=Dn[:, :], func=AF.Ln)
            nc.scalar.activation(out=R[:, :], in_=LD[:, :], func=AF.Exp,
                                 scale=-1.0)
            # step2: merge pairs into two sorted-4 lists A, B
            mi0 = _slots(S, 0, 2, 2)[:, :, 0, :]      # m0, m2
            mi1 = _slots(S, 0, 2, 2)[:, :, 1, :]      # m1, m3
            ni0 = _slots(S, 3200, 2, 2)[:, :, 0, :]   # n0, n2
            ni1 = _slots(S, 3200, 2, 2)[:, :, 1, :]   # n1, n3
            nc.vector.tensor_tensor(  # A0, B0
                out=_slots(T, 0, 2, 2)[:, :, 0, :], in0=mi0, in1=mi1,
                op=ALU.max)
            nc.vector.tensor_tensor(  # x, x'
                out=_slots(T, 3200, 2, 2)[:, :, 0, :], in0=mi0, in1=mi1,
                op=ALU.min)
            nc.vector.tensor_tensor(  # y, y'
                out=_slots(T, 3200, 2, 2)[:, :, 1, :], in0=ni0, in1=ni1,
                op=ALU.max)
            nc.vector.tensor_tensor(  # A3, B3
                out=_slots(T, 0, 2, 2)[:, :, 1, :], in0=ni0, in1=ni1,
                op=ALU.min)
            xi0 = _slots(T, 3200, 2, 2)[:, :, 0, :]   # x, x'
            xi1 = _slots(T, 3200, 2, 2)[:, :, 1, :]   # y, y'
            nc.vector.tensor_tensor(  # A1, B1
                out=_slots(S, 0, 2, 2)[:, :, 0, :], in0=xi0, in1=xi1,
                op=ALU.max)
            nc.vector.tensor_tensor(  # A2, B2
                out=_slots(S, 0, 2, 2)[:, :, 1, :], in0=xi0, in1=xi1,
                op=ALU.min)
            # lists: A = [T0, S0, S1, T1],  B = [T2, S2, S3, T3]
            # step3: bitonic top-4  c_i = max(A_i, B_{3-i})
            nc.vector.tensor_tensor(out=_sl(Cc, 0), in0=_sl(T, 0),
                                    in1=_sl(T, 3), op=ALU.max)
            nc.vector.tensor_tensor(out=_sl(Cc, 1), in0=_sl(S, 0),
                                    in1=_sl(S, 3), op=ALU.max)
            nc.vector.tensor_tensor(out=_sl(Cc, 2), in0=_sl(S, 1),
                                    in1=_sl(S, 2), op=ALU.max)
            nc.vector.tensor_tensor(out=_sl(Cc, 3), in0=_sl(T, 1),
                                    in1=_sl(T, 2), op=ALU.max)
            # step4: bitonic sort of (c0..c3) descending
            ci0 = Cc[:, 0:1600].rearrange("q (a gf) -> q a gf", a=2, gf=GF)
            ci1 = Cc[:, 1600:3200].rearrange("q (a gf) -> q a gf", a=2, gf=GF)
            do0 = Dd[:, 0:1600].rearrange("q (a gf) -> q a gf", a=2, gf=GF)
            do1 = Dd[:, 1600:3200].rearrange("q (a gf) -> q a gf", a=2, gf=GF)
            nc.vector.tensor_tensor(out=do0, in0=ci0, in1=ci1, op=ALU.max)
            nc.vector.tensor_tensor(out=do1, in0=ci0, in1=ci1, op=ALU.min)
            ST_g = ST[:, 0:3200].rearrange("q (g jj f) -> q g jj f",
                                           g=4, jj=4, f=F)

            def dsl(k):
                return _sl(Dd, k).rearrange("q (g f) -> q g f", g=4, f=F)

            nc.vector.tensor_tensor(out=ST_g[:, :, 0, :], in0=dsl(0),
                                    in1=dsl(1), op=ALU.max)
            nc.vector.tensor_tensor(out=ST_g[:, :, 1, :], in0=dsl(0),
                                    in1=dsl(1), op=ALU.min)
            nc.vector.tensor_tensor(out=ST_g[:, :, 2, :], in0=dsl(2),
                                    in1=dsl(3), op=ALU.max)
            nc.vector.tensor_tensor(out=ST_g[:, :, 3, :], in0=dsl(2),
                                    in1=dsl(3), op=ALU.min)
            # scale by softmax reciprocal
            Rv = R[:, :].rearrange("q (g f) -> q g f", g=4, f=F)
            for jj in range(4):
                nc.vector.tensor_tensor(out=ST_g[:, :, jj, :],
                                        in0=ST_g[:, :, jj, :], in1=Rv,
                                        op=ALU.mult)

            # ---- transposes + MLP, in groups of up to 4 rounds ----
            ST_ch = ST[:, :].rearrange("q (c f) -> q c f", c=32, f=F)
            y2p = y2_ps.tile([128, F], F32, tag="y2")
            n_rounds = F // 4  # 50
            grp_starts = list(range(0, n_rounds, 4))
            for grp, r0 in enumerate(grp_starts):
                rg = min(4, n_rounds - r0)
                nf = 128 * rg
                pt = pt_ps.tile([128, nf], BF16, tag="pt")
                for r4 in range(rg):
                    for j in range(4):
                        f = 4 * (r0 + r4) + j
                        nc.tensor.matmul(
                            out=pt[32 * j:32 * j + 32, 128 * r4:128 * r4 + 128],
                            lhsT=ST_ch[:, :, f], rhs=idt[:, :],
                            is_transpose=True, tile_position=(0, 32 * j),
                            start=True, stop=True)
                mb = mpool.tile([128, nf], BF16, tag="mb")
                nc.scalar.activation(out=mb[:, :], in_=pt[:, :], func=AF.Copy)
                for b in range(2):
                    yp = yp_ps.tile([128, nf], F32, tag="yp")
                    for ii in range(2):
                        i = 2 * b + ii
                        for jo in range(2):
                            jc = 2 * ii + jo
                            nc.tensor.matmul(
                                out=yp[32 * jc:32 * jc + 32, :],
                                lhsT=w1s[32 * i:32 * i + 32,
                                         32 * jo:32 * jo + 32],
                                rhs=mb[32 * i:32 * i + 32, :],
                                tile_position=(32 * i, 32 * jc),
                                start=True, stop=True)
                    z = zpool.tile([128, nf], BF16, tag="z")
                    if b == 0:
                        nc.scalar.activation(out=z[:, :], in_=yp[:, :],
                                             func=AF.Relu, bias=b1s[:, 0:1])
                    else:
                        nc.vector.tensor_scalar(
                            out=z[:, :], in0=yp[:, :], scalar1=b1s[:, 0:1],
                            scalar2=0.0, op0=ALU.add, op1=ALU.max)
                    for c in range(rg):
                        off = 16 * grp + 4 * c + 2 * b
                        nc.tensor.matmul(
                            out=y2p[:, off:off + 2],
                            lhsT=z[:, 128 * c:128 * c + 128],
                            rhs=w2c[:, :], start=True, stop=True)

            y2s = opool.tile([128, F], F32, tag="y2s")
            nc.scalar.activation(out=y2s[:, :], in_=y2p[:, :],
                                 func=AF.Sigmoid, bias=b2s[:, 0:1])
            nc.sync.dma_start(
                out=y_d[n].rearrange("(q f) -> q f", q=Q, f=F),
                in_=y2s[:, :])
    if legalize:
        _legalize_sync_waits(nc)
    return nc


def _legalize_sync_waits(nc):
    """Walrus rejects instructions with too many semaphore waits
    ("Too many sync wait commands"). Spill excess waits onto a
    same-engine Drain inserted right before the offending instruction.
    HWDGE DMA descriptors fit a single wait; compute instructions two."""
    k = 0
    for blk in nc.m.functions[0].blocks:
        insts = blk.instructions
        out = []
        for inst in insts:
            ty = type(inst).__name__
            if ty in ("InstCall", "InstUnconditionalBranch"):
                out.append(inst)
                continue
            limit = 1
            si = inst.sync_info
            if si is not None and si.on_wait and len(si.on_wait) > limit:
                waits = list(si.on_wait)
                for w in waits[:-limit]:
                    d = mybir.InstDrain(name=f"W-spill-{k}",
                                        engine=inst.engine)
                    k += 1
                    d.sync_info = mybir.SyncInfo(on_wait=[w], on_update=[])
                    out.append(d)
                inst.sync_info = mybir.SyncInfo(
                    on_wait=waits[-limit:], on_update=list(si.on_update))
            out.append(inst)
        if k:
            blk.instructions = out


def prep_consts(w1, b1, w2, b2):
    bf = ml_dtypes.bfloat16
    w1 = np.asarray(w1, np.float32).reshape(64, 4, 5)
    wf = (w1[:, :, :4] + 0.25 * w1[:, :, 4:5]).reshape(64, 16)  # och x ch
    blk = np.zeros((32, 64), np.float32)
    blk[:16, :] = wf.T
    w1s = np.tile(blk, (4, 1)).astype(bf)                       # (128, 64)
    w2 = np.asarray(w2, np.float32).reshape(64)
    w2c = np.zeros((128, 2), np.float32)
    w2c[:64, 0] = w2
    w2c[64:, 1] = w2
    b1 = np.asarray(b1, np.float32).reshape(64)
    b1s = np.tile(b1, 2).reshape(128, 1).astype(np.float32)
    b2s = np.full((128, 1), np.asarray(b2, np.float32).reshape(-1)[0],
                  np.float32)
    idt = np.eye(128, dtype=np.float32).astype(bf)
    return {"w1s": w1s, "w2c": w2c.astype(bf), "b1s": b1s, "b2s": b2s,
            "idt": idt}


_CACHE = {}


def _get_nc(n_img=N_PER):
    if n_img not in _CACHE:
        _CACHE[n_img] = build_bass(n_img)
    return _CACHE[n_img]


def _ensure_ntff_hook():
    """Provide antenv.axon_hooks if the image lacks it (profiling only)."""
    import sys
    import types
    try:
        from antenv.axon_hooks import get_axon_ntff_profile_hook  # noqa: F401
        return
    except ImportError:
        pass
    try:
        import antenv
        from trn_agent_boot.trn_boot import _ntff_profile_via_ctypes
        hook = _ntff_profile_via_ctypes("/opt/axon/libaxon_pjrt.so")
        mod = types.ModuleType("antenv.axon_hooks")
        mod._hook = hook
        mod.get_axon_ntff_profile_hook = lambda: mod._hook
        mod.set_axon_ntff_profile_hook = lambda h: setattr(mod, "_hook", h)
        sys.modules["antenv.axon_hooks"] = mod
        antenv.axon_hooks = mod
    except Exception as e:  # profiling is best-effort
        print(f"ntff hook setup failed: {e}")


def run_cores(x, consts, trace=False):
    """x: (32, 32, 25600) f32 -> (32, 25600) f32 via 8-core SPMD."""
    if trace:
        _ensure_ntff_hook()
    nc = _get_nc()
    xs = np.ascontiguousarray(x, np.float32).reshape(N_CORES, N_PER, C_IN, HW)
    in_maps = [dict(consts, x=xs[k]) for k in range(N_CORES)]
    res = bass_utils.run_bass_kernel_spmd(
        nc, in_maps, core_ids=list(range(N_CORES)), trace=trace)
    y = np.stack([res.results[k]["y"] for k in range(N_CORES)])
    return y.reshape(N_CORES * N_PER, HW), res


def kernel(x, w1, b1, w2, b2):
    N, C, H, W = x.shape
    consts = prep_consts(w1, b1, w2, b2)
    y, _ = run_cores(np.asarray(x, np.float32).reshape(N, C, H * W), consts)
    return y.reshape(N, 1, H, W).astype(np.float32)



# revision 7
# speedup vs baseline: 1.9480x; 1.9480x over previous
"""DGQP (distribution-guided quality predictor) Trainium2 Bass kernel, v2.

Full-input contract: kernel(**inputs) takes the unsharded inputs from
setup_inputs() and returns the full (32, 1, 160, 160) float32 output.
Internally: pure data parallel over 8 NeuronCores (4 images each).

Per-image pipeline on one core (H*W = 25600 = 128 q-blocks x 200 f):
  DMA     x[n] (32, 25600) -> SBUF X [128 part=q, free = c*200 + f]
  ACT     E = exp(X)  (bf16)  -- softmax without max-subtract (|x| ~ N(0,1))
  DVE     top-4-of-8 sorted per (g, q, f) compare-exchange network (bf16 2x)
  DVE     denominator add-tree (3 bf16 ops, reuses step-1 pair max/min)
  ACT     R = exp(-ln(Dn))    (reciprocal via ln/exp)
  DVE     ST = top4 * R  (scaled stats, ch = 4g+jj at stride 200)
  PE      batched stat transposes: per 8-pixel tile one [128,(f8,c)=128]
          is_transpose matmul -> PT psum [(f8,c), q]
  DVE     mb = copy(PT) psum->sbuf bf16 (batched 4 tiles per copy)
  PE      mm1: YP_b = W1blk_b.T @ mb  (block-diag weights, och-chunked,
          b-outer ordering for stationary reuse)
  ACT     z = relu(YP + b1) psum->sbuf bf16 (batched 10-tile regions)
  PE      mm2: Y2[:, 8t:8t+8] += z_tb.T @ w2cols_b  (4-step psum accum)
  ACT     sigmoid(Y2 + b2) -> SBUF [128 q, 200 f]
  DMA     -> y[n] (25600,)
"""

import numpy as np
import ml_dtypes
from contextlib import ExitStack

import concourse.bass as bass
import concourse.mybir as mybir
from concourse.tile import TileContext
from concourse import bass_utils

F32 = mybir.dt.float32
BF16 = mybir.dt.bfloat16
AX = mybir.AxisListType
ALU = mybir.AluOpType
AF = mybir.ActivationFunctionType

N_CORES = 8
N_PER = 4          # images per core
C_IN = 32          # channels = 4 groups x 8 bins
HW = 25600         # 160*160
Q = 128            # partition blocks
F = 200            # pixels per partition per image
GF = 800           # 4 groups * 200


def _sl(t, k, w=GF):
    """Flat slice k of width w in the free dim."""
    return t[:, k * w:(k + 1) * w]


def _slots(t, off, a, s, gf=GF):
    """View t[:, off:off+a*s*gf] as (a, s, gf): slot picks with stride a*s."""
    return t[:, off:off + a * s * gf].rearrange(
        "q (a s gf) -> q a s gf", a=a, s=s, gf=gf)


def build_bass(n_img=N_PER, legalize=True):
    nc = bass.Bass("TRN2", target_bir_lowering=False, debug=False)

    x_d = nc.dram_tensor("x", (n_img, C_IN, HW), F32, kind="ExternalInput").ap()
    w1b_d = nc.dram_tensor("w1b", (128, 512), BF16, kind="ExternalInput").ap()
    w2c_d = nc.dram_tensor("w2c", (128, 32), BF16, kind="ExternalInput").ap()
    b1v_d = nc.dram_tensor("b1v", (128, 4), F32, kind="ExternalInput").ap()
    b2s_d = nc.dram_tensor("b2s", (128, 1), F32, kind="ExternalInput").ap()
    idt_d = nc.dram_tensor("idt", (128, 128), BF16, kind="ExternalInput").ap()
    y_d = nc.dram_tensor("y", (n_img, HW), F32, kind="ExternalOutput").ap()

    NT = 25            # f8-tiles per image
    WV = 8             # f8-tiles per z-wave
    NW = 4             # waves (8, 8, 8, 1)

    with TileContext(nc) as tc, ExitStack() as ctx:
        cpool = ctx.enter_context(tc.tile_pool(name="const", bufs=1))
        xpool = ctx.enter_context(tc.tile_pool(name="xin", bufs=2))
        epool = ctx.enter_context(tc.tile_pool(name="exp", bufs=2))
        spool = ctx.enter_context(tc.tile_pool(name="sort", bufs=1))
        stpool = ctx.enter_context(tc.tile_pool(name="st", bufs=1))
        ctpool = ctx.enter_context(tc.tile_pool(name="ct", bufs=2))
        dpool = ctx.enter_context(tc.tile_pool(name="den", bufs=2))
        mpool = ctx.enter_context(tc.tile_pool(name="mb", bufs=2))
        zpool = ctx.enter_context(tc.tile_pool(name="z", bufs=1))
        opool = ctx.enter_context(tc.tile_pool(name="out", bufs=2))
        pt_ps = ctx.enter_context(tc.tile_pool(name="pt", bufs=2, space="PSUM"))
        yp_ps = ctx.enter_context(tc.tile_pool(name="yp", bufs=2, space="PSUM"))
        y2_ps = ctx.enter_context(tc.tile_pool(name="y2", bufs=2, space="PSUM"))

        w1b = cpool.tile([128, 512], BF16, tag="w1b")
        w2c = cpool.tile([128, 32], BF16, tag="w2c")
        b1v = cpool.tile([128, 4], F32, tag="b1v")
        b2s = cpool.tile([128, 1], F32, tag="b2s")
        idt = cpool.tile([128, 128], BF16, tag="idt")
        nc.sync.dma_start(out=w1b[:, :], in_=w1b_d)
        nc.sync.dma_start(out=w2c[:, :], in_=w2c_d)
        nc.sync.dma_start(out=b1v[:, :], in_=b1v_d)
        nc.sync.dma_start(out=b2s[:, :], in_=b2s_d)
        nc.sync.dma_start(out=idt[:, :], in_=idt_d)

        for n in range(n_img):
            # X/E free layout is c-major: c*200 + f with c = 8g + j,
            # so the input is a single 3-dim DMA (contiguous 800B runs).
            X = xpool.tile([128, 6400], F32, tag="x")
            nc.sync.dma_start(
                out=X[:, :].rearrange("q (c f) -> q c f", c=32, f=F),
                in_=x_d[n].rearrange("c (q f) -> q c f", q=Q, f=F))

            # exp of all logits (order-preserving for the sort; feeds denom)
            E = epool.tile([128, 6400], BF16, tag="e")
            nc.scalar.activation(out=E[:, :], in_=X[:, :], func=AF.Exp)

            # ---- top-4-of-8 sorting network (bf16 slices of 800) ----
            S = spool.tile([128, 6400], BF16, tag="s")
            T = spool.tile([128, 6400], BF16, tag="t")
            Cc = spool.tile([128, 3200], BF16, tag="c")
            ST = stpool.tile([128, 3200], BF16, tag="st")
            Dd = T[:, 0:3200]  # step-4 scratch (T dead after step3)

            # E slice for bin j of group g sits at (g*8 + j)*200; scratch
            # tiles S/T/C keep slot-major layout (slot*800, g*200+f inside).
            Ev = E[:, :].rearrange("q (g p two f) -> q p two g f",
                                   g=4, p=4, two=2, f=F)
            S04 = S[:, 0:3200].rearrange("q (a g f) -> q a g f",
                                         a=4, g=4, f=F)
            S48 = S[:, 3200:6400].rearrange("q (a g f) -> q a g f",
                                            a=4, g=4, f=F)
            # step1: sorted pairs  m_p = max, n_p = min
            nc.vector.tensor_tensor(out=S04, in0=Ev[:, :, 0],
                                    in1=Ev[:, :, 1], op=ALU.max)
            nc.vector.tensor_tensor(out=S48, in0=Ev[:, :, 0],
                                    in1=Ev[:, :, 1], op=ALU.min)
            # denominator add-tree on DVE (bf16, reads step-1 pair sums)
            P2 = dpool.tile([128, 3200], BF16, tag="p2")
            Dn = dpool.tile([128, GF], BF16, tag="dn")
            LD = dpool.tile([128, GF], F32, tag="ld")
            R = dpool.tile([128, GF], BF16, tag="r")
            nc.vector.tensor_tensor(out=P2[:, :], in0=S[:, 0:3200],
                                    in1=S[:, 3200:6400], op=ALU.add)
            nc.vector.tensor_tensor(out=P2[:, 0:1600], in0=P2[:, 0:1600],
                                    in1=P2[:, 1600:3200], op=ALU.add)
            nc.vector.tensor_tensor(out=Dn[:, :], in0=P2[:, 0:GF],
                                    in1=P2[:, GF:1600], op=ALU.add)
            nc.scalar.activation(out=LD[:, :], in_=Dn[:, :], func=AF.Ln)
            nc.scalar.activation(out=R[:, :], in_=LD[:, :], func=AF.Exp,
                                 scale=-1.0)
            # step2: merge pairs into two sorted-4 lists A, B
            mi0 = _slots(S, 0, 2, 2)[:, :, 0, :]      # m0, m2
            mi1 = _slots(S, 0, 2, 2)[:, :, 1, :]      # m1, m3
            ni0 = _slots(S, 3200, 2, 2)[:, :, 0, :]   # n0, n2
            ni1 = _slots(S, 3200, 2, 2)[:, :, 1, :]   # n1, n3
            nc.vector.tensor_tensor(  # A0, B0
                out=_slots(T, 0, 2, 2)[:, :, 0, :], in0=mi0, in1=mi1,
                op=ALU.max)
            nc.vector.tensor_tensor(  # x, x'
                out=_slots(T, 3200, 2, 2)[:, :, 0, :], in0=mi0, in1=mi1,
                op=ALU.min)
            nc.vector.tensor_tensor(  # y, y'
                out=_slots(T, 3200, 2, 2)[:, :, 1, :], in0=ni0, in1=ni1,
                op=ALU.max)
            nc.vector.tensor_tensor(  # A3, B3
                out=_slots(T, 0, 2, 2)[:, :, 1, :], in0=ni0, in1=ni1,
                op=ALU.min)
            xi0 = _slots(T, 3200, 2, 2)[:, :, 0, :]   # x, x'
            xi1 = _slots(T, 3200, 2, 2)[:, :, 1, :]   # y, y'
            nc.vector.tensor_tensor(  # A1, B1
                out=_slots(S, 0, 2, 2)[:, :, 0, :], in0=xi0, in1=xi1,
                op=ALU.max)
            nc.vector.tensor_tensor(  # A2, B2
                out=_slots(S, 0, 2, 2)[:, :, 1, :], in0=xi0, in1=xi1,
                op=ALU.min)
            # lists: A = [T0, S0, S1, T1],  B = [T2, S2, S3, T3]
            # step3: bitonic top-4  c_i = max(A_i, B_{3-i})
            nc.vector.tensor_tensor(out=_sl(Cc, 0), in0=_sl(T, 0),
                                    in1=_sl(T, 3), op=ALU.max)
            nc.vector.tensor_tensor(out=_sl(Cc, 1), in0=_sl(S, 0),
                                    in1=_sl(S, 3), op=ALU.max)
            nc.vector.tensor_tensor(out=_sl(Cc, 2), in0=_sl(S, 1),
                                    in1=_sl(S, 2), op=ALU.max)
            nc.vector.tensor_tensor(out=_sl(Cc, 3), in0=_sl(T, 1),
                                    in1=_sl(T, 2), op=ALU.max)
            # step4: bitonic sort of (c0..c3) descending
            ci0 = Cc[:, 0:1600].rearrange("q (a gf) -> q a gf", a=2, gf=GF)
            ci1 = Cc[:, 1600:3200].rearrange("q (a gf) -> q a gf", a=2, gf=GF)
            do0 = Dd[:, 0:1600].rearrange("q (a gf) -> q a gf", a=2, gf=GF)
            do1 = Dd[:, 1600:3200].rearrange("q (a gf) -> q a gf", a=2, gf=GF)
            nc.vector.tensor_tensor(out=do0, in0=ci0, in1=ci1, op=ALU.max)
            nc.vector.tensor_tensor(out=do1, in0=ci0, in1=ci1, op=ALU.min)
            ST_g = ST[:, :].rearrange("q (g jj f) -> q g jj f",
                                      g=4, jj=4, f=F)

            def dsl(k):
                return _sl(Dd, k).rearrange("q (g f) -> q g f", g=4, f=F)

            nc.vector.tensor_tensor(out=ST_g[:, :, 0, :], in0=dsl(0),
                                    in1=dsl(1), op=ALU.max)
            nc.vector.tensor_tensor(out=ST_g[:, :, 1, :], in0=dsl(0),
                                    in1=dsl(1), op=ALU.min)
            nc.vector.tensor_tensor(out=ST_g[:, :, 2, :], in0=dsl(2),
                                    in1=dsl(3), op=ALU.max)
            nc.vector.tensor_tensor(out=ST_g[:, :, 3, :], in0=dsl(2),
                                    in1=dsl(3), op=ALU.min)
            # scale by softmax reciprocal
            Rv = R[:, :].rearrange("q (g f) -> q g f", g=4, f=F)
            for jj in range(4):
                nc.vector.tensor_tensor(out=ST_g[:, :, jj, :],
                                        in0=ST_g[:, :, jj, :], in1=Rv,
                                        op=ALU.mult)

            # ---- reorder stats tile-contiguous, then batched transposes ----
            # ST free layout is ch*200 + f (ch = 4g+jj, f = 8t + f8).
            # CT packs each 8-pixel tile t contiguously with k = 8c + f8,
            # so the transpose stationary is a plain 128-wide slice.
            CT = ctpool.tile([128, 3200], BF16, tag="ct")
            nc.vector.tensor_copy(
                out=CT[:, :],
                in_=ST[:, :].rearrange("q (c t f8) -> q t c f8",
                                       c=16, t=NT, f8=8))
            mb = mpool.tile([128, 3200], BF16, tag="mb")
            for w in range(7):          # 4 f8-tiles per PT psum tile
                t0 = 4 * w
                ntile = min(4, NT - t0)
                nf = 128 * ntile
                pt = pt_ps.tile([128, 512], BF16, tag="pt")
                for k in range(ntile):
                    t = t0 + k
                    nc.tensor.matmul(
                        out=pt[:, 128 * k:128 * k + 128],
                        lhsT=CT[:, 128 * t:128 * t + 128], rhs=idt[:, :],
                        is_transpose=True, start=True, stop=True)
                nc.vector.tensor_copy(out=mb[:, 512 * w:512 * w + nf],
                                      in_=pt[:, 0:nf])

            # ---- mm1 (b-outer for stationary reuse) + relu waves ----
            z_all = zpool.tile([128, 12800], BF16, tag="z")
            for b in range(4):
                for w in range(NW):
                    t0 = WV * w
                    ntile = min(WV, NT - t0)
                    nf = 128 * ntile
                    yp = yp_ps.tile([128, 1024], F32, tag="yp")
                    for k in range(ntile):
                        t = t0 + k
                        nc.tensor.matmul(
                            out=yp[:, 128 * k:128 * k + 128],
                            lhsT=w1b[:, 128 * b:128 * b + 128],
                            rhs=mb[:, 128 * t:128 * t + 128],
                            start=True, stop=True)
                    nc.scalar.activation(
                        out=z_all[:, 3200 * b + 128 * t0:
                                  3200 * b + 128 * t0 + nf],
                        in_=yp[:, 0:nf], func=AF.Relu,
                        bias=b1v[:, b:b + 1])

            # ---- mm2: 4-chunk psum accumulation per f8-tile ----
            y2p = y2_ps.tile([128, F], F32, tag="y2")
            for t in range(NT):
                for b in range(4):
                    nc.tensor.matmul(
                        out=y2p[:, 8 * t:8 * t + 8],
                        lhsT=z_all[:, 3200 * b + 128 * t:
                                   3200 * b + 128 * t + 128],
                        rhs=w2c[:, 8 * b:8 * b + 8],
                        start=(b == 0), stop=(b == 3))

            y2s = opool.tile([128, F], F32, tag="y2s")
            nc.scalar.activation(out=y2s[:, :], in_=y2p[:, :],
                                 func=AF.Sigmoid, bias=b2s[:, 0:1])
            nc.sync.dma_start(
                out=y_d[n].rearrange("(q f) -> q f", q=Q, f=F),
                in_=y2s[:, :])
    if legalize:
        _legalize_sync_waits(nc)
    return nc


def _legalize_sync_waits(nc):
    """Walrus rejects instructions with too many semaphore waits
    ("Too many sync wait commands"). Spill excess waits onto a
    same-engine Drain inserted right before the offending instruction.
    HWDGE DMA descriptors fit a single wait; compute instructions two."""
    k = 0
    for blk in nc.m.functions[0].blocks:
        insts = blk.instructions
        out = []
        for inst in insts:
            ty = type(inst).__name__
            if ty in ("InstCall", "InstUnconditionalBranch"):
                out.append(inst)
                continue
            limit = 1
            si = inst.sync_info
            if si is not None and si.on_wait and len(si.on_wait) > limit:
                waits = list(si.on_wait)
                for w in waits[:-limit]:
                    d = mybir.InstDrain(name=f"W-spill-{k}",
                                        engine=inst.engine)
                    k += 1
                    d.sync_info = mybir.SyncInfo(on_wait=[w], on_update=[])
                    out.append(d)
                inst.sync_info = mybir.SyncInfo(
                    on_wait=waits[-limit:], on_update=list(si.on_update))
            out.append(inst)
        if k:
            blk.instructions = out
    return nc


def prep_consts(w1, b1, w2, b2):
    bf = ml_dtypes.bfloat16
    w1 = np.asarray(w1, np.float32).reshape(64, 4, 5)
    wf = (w1[:, :, :4] + 0.25 * w1[:, :, 4:5]).reshape(64, 16)  # och x ch
    w2 = np.asarray(w2, np.float32).reshape(64)
    b1 = np.asarray(b1, np.float32).reshape(64)
    # w1b: 4 blocks [128, 128]; rows p = 8c + f8, cols j = 16f8' + ob:
    # w1b_b[8c+f8, 16f8+ob] = wf[16b+ob, c]
    w1b = np.zeros((128, 512), np.float32)
    for b in range(4):
        for f8 in range(8):
            for c in range(16):
                w1b[8 * c + f8, 128 * b + 16 * f8:128 * b + 16 * f8 + 16] = \
                    wf[16 * b:16 * b + 16, c]
    # w2c: 4 blocks [128, 8]; block b: [16f8+ob, f8] = w2[16b+ob]
    w2c = np.zeros((128, 32), np.float32)
    for b in range(4):
        for f8 in range(8):
            w2c[16 * f8:16 * f8 + 16, 8 * b + f8] = w2[16 * b:16 * b + 16]
    # b1v: column b = tile(b1[16b:16b+16], 8)
    b1v = np.zeros((128, 4), np.float32)
    for b in range(4):
        b1v[:, b] = np.tile(b1[16 * b:16 * b + 16], 8)
    b2s = np.full((128, 1), np.asarray(b2, np.float32).reshape(-1)[0],
                  np.float32)
    idt = np.eye(128, dtype=np.float32).astype(bf)
    return {"w1b": w1b.astype(bf), "w2c": w2c.astype(bf),
            "b1v": b1v, "b2s": b2s, "idt": idt}


_CACHE = {}


def _get_nc(n_img=N_PER):
    if n_img not in _CACHE:
        _CACHE[n_img] = build_bass(n_img)
    return _CACHE[n_img]


def _ensure_ntff_hook():
    """Provide antenv.axon_hooks if the image lacks it (profiling only)."""
    import sys
    import types
    try:
        from antenv.axon_hooks import get_axon_ntff_profile_hook  # noqa: F401
        return
    except ImportError:
        pass
    try:
        import antenv
        from trn_agent_boot.trn_boot import _ntff_profile_via_ctypes
        hook = _ntff_profile_via_ctypes("/opt/axon/libaxon_pjrt.so")
        mod = types.ModuleType("antenv.axon_hooks")
        mod._hook = hook
        mod.get_axon_ntff_profile_hook = lambda: mod._hook
        mod.set_axon_ntff_profile_hook = lambda h: setattr(mod, "_hook", h)
        sys.modules["antenv.axon_hooks"] = mod
        antenv.axon_hooks = mod
    except Exception as e:  # profiling is best-effort
        print(f"ntff hook setup failed: {e}")


def run_cores(x, consts, trace=False):
    """x: (32, 32, 25600) f32 -> (32, 25600) f32 via 8-core SPMD."""
    if trace:
        _ensure_ntff_hook()
    nc = _get_nc()
    xs = np.ascontiguousarray(x, np.float32).reshape(N_CORES, N_PER, C_IN, HW)
    in_maps = [dict(consts, x=xs[k]) for k in range(N_CORES)]
    res = bass_utils.run_bass_kernel_spmd(
        nc, in_maps, core_ids=list(range(N_CORES)), trace=trace)
    y = np.stack([res.results[k]["y"] for k in range(N_CORES)])
    return y.reshape(N_CORES * N_PER, HW), res


def kernel(x, w1, b1, w2, b2):
    N, C, H, W = x.shape
    consts = prep_consts(w1, b1, w2, b2)
    y, _ = run_cores(np.asarray(x, np.float32).reshape(N, C, H * W), consts)
    return y.reshape(N, 1, H, W).astype(np.float32)
